# revision 29
# baseline (speedup 1.0000x reference)
"""BlockSparseDilatedAttention TRN2 kernel (v2).

Full inputs q,k,v: [1, 8192, 12, 64] fp32. Output: same shape.

Math: 16 blocks of 512 tokens; block pairs (r, c) with |r-c| <= 2 (74 pairs).
Per pair, dilated segment attention in 3 head-groups of 4 heads:
  g0: seg 128, dil 1 -> 4 units of 128 tokens per block
  g1: seg 256, dil 2 -> 2 units of 128 (odd positions)
  g2: seg 128, dil 4 -> 1 unit of 128 (pos 2 mod 4), block-diag mask of 4x32
Each unit: softmax(Q K^T / 8) V over its own 128 kv tokens, normalized PER
BLOCK PAIR; pair outputs are scatter-added into the query row block.

Sharding: 8 cores = 2 head-halves x 4 row-quarters. Identical SPMD program;
edge cores get zero-padded kv blocks (zero V => zero contribution).

Device pipeline per (group, u-sweep, col-block):
  S^T = matmul(K^T stationary, Q^T moving)  -> 2-bank PSUM tile (h-pair
    concurrent on disjoint PE row groups; NB: concurrent matmuls must
    write different PSUM banks - same-bank is a fatal HW collision)
  A^T = exp(S^T/8)  (ScalarE, one [128,2,nr*128] ACTIVATE per (cb,u))
  po  = matmul(A^T stationary, [V | 1] moving) per (h,row,delta) -> [q,65]
  normalize (per block pair): recip (DVE custom), per-delta mul by the
    broadcast reciprocals (DVE), then a 3-op pairwise add-tree over the
    5 deltas (GpSimd in f32 for g0/g1, DVE in f16 for g2)
  outputs land in SBUF staging [128, i, h, u, 64] f32; one DMA per
    (group, row-half).
Input DMA: one HWDGE ring (sync), ~23 descriptors with 1-2KB lines,
ordered by first use (scalar-ring and gpsimd SWDGE variants measured
slower - DMA issue occupancy delays the pacing ACTIVATE queue).
"""

import sys

import numpy as np

for _p in ("/opt/trn_rl_repo",):
    if _p not in sys.path:
        sys.path.append(_p)

# ---------------------------------------------------------------- constants
B, S, H, D = 1, 8192, 12, 64
BLOCK = 512
NB = S // BLOCK            # 16
GL = [512, 256, 128]       # gathered tokens per block, per group
GNU = [4, 2, 1]            # 128-token units per block, per group
GC = [64, 64, 69]          # contraction rows (g2 has 5 mask-aug rows)
MASK_M = 512.0
NCORES = 8
ROWS_PER_CORE = 4          # row blocks per quarter
CB = 8                     # col blocks per core (4 rows window, padded)
SCALE = 0.125              # 1/sqrt(64)

S_DTYPE = "f16"
AV_DTYPE = "f16"

def _gather_pos():
    pos = [np.arange(512)]
    pos.append(np.concatenate([s + 1 + 2 * np.arange(128) for s in (0, 256)]))
    pos.append(np.concatenate([s + 2 + 4 * np.arange(32) for s in (0, 128, 256, 384)]))
    return pos


POS = _gather_pos()


def _rows_for_cb(cb):
    """Local row indices i in [0,3] attending col block cb (c = 4R-2+cb)."""
    return max(0, cb - 4), min(3, cb)


# ---------------------------------------------------------------- bass build
_BASS_CACHE = {}


def _build_bass():
    if "nc" in _BASS_CACHE:
        return _BASS_CACHE["nc"]

    import concourse.tile as tile
    from concourse import bacc, mybir

    exp_fn = mybir.ActivationFunctionType.Exp
    add_op = mybir.AluOpType.add
    f32 = mybir.dt.float32
    dt_map = {"f32": f32, "f32r": mybir.dt.float32r, "bf16": mybir.dt.bfloat16,
              "f16": mybir.dt.float16}
    sdt = dt_map[S_DTYPE]
    avdt = dt_map[AV_DTYPE]
    nc = bacc.Bacc("TRN2", target_bir_lowering=False, debug=False,
                   enable_asserts=False)

    qt_d, kt_d, v_d, out_d = [], [], [], []
    for g in range(3):
        L, nu, C = GL[g], GNU[g], GC[g]
        qp = 128 if g < 2 else C
        qf = 4 * L if g < 2 else 2 * 4 * L
        kf = CB * L if g < 2 else 2 * CB * L
        qt_d.append(nc.dram_tensor(f"qt{g}", [qp, qf], sdt, kind="ExternalInput").ap())
        kt_d.append(nc.dram_tensor(f"kt{g}", [qp, kf], sdt, kind="ExternalInput").ap())
        v_d.append(nc.dram_tensor(f"v{g}", [128, 2 * CB * nu * 66], avdt,
                                  kind="ExternalInput").ap())
        out_d.append(nc.dram_tensor(f"out{g}", [128, 4 * 2 * nu * 64], f32,
                                    kind="ExternalOutput").ap())

    with tile.TileContext(nc) as tc:
        with tc.tile_pool(name="inp", bufs=1) as inp, \
             tc.tile_pool(name="atp", bufs=42) as atp, \
             tc.tile_pool(name="rcp", bufs=8) as rcp, \
             tc.tile_pool(name="tmpp", bufs=4) as tmpp, \
             tc.tile_pool(name="trp", bufs=6) as trp, \
             tc.tile_pool(name="ps_s", bufs=2, space="PSUM") as ps_s, \
             tc.tile_pool(name="ps_o", bufs=2, space="PSUM") as ps_o:

            # ---------------- inputs + their DMA plan (sync + scalar rings)
            qt_sb, kt_sb, v_sb = [], [], []
            for g in range(3):
                qt_sb.append(inp.tile(list(qt_d[g].shape), sdt, tag=f"qt{g}", name=f"qt{g}"))
                kt_sb.append(inp.tile(list(kt_d[g].shape), sdt, tag=f"kt{g}", name=f"kt{g}"))
                v_sb.append(inp.tile(list(v_d[g].shape), avdt, tag=f"v{g}", name=f"v{g}"))

            def chunks(sb, dr, n):
                w = dr.shape[1]
                step = (w // n + 63) // 64 * 64
                out = []
                p = 0
                while p < w:
                    e = min(w, p + step)
                    out.append((sb[:, p:e], dr[:, p:e]))
                    p = e
                return out

            sync_loads = (chunks(kt_sb[0], kt_d[0], 4)[:1]
                          + chunks(qt_sb[0], qt_d[0], 2)[:1]
                          + chunks(kt_sb[0], kt_d[0], 4)[1:2]
                          + chunks(qt_sb[0], qt_d[0], 2)[1:]
                          + chunks(v_sb[0], v_d[0], 4)[:2]
                          + chunks(kt_sb[0], kt_d[0], 4)[2:]
                          + chunks(v_sb[0], v_d[0], 4)[2:]
                          + chunks(kt_sb[1], kt_d[1], 2)
                          + chunks(qt_sb[1], qt_d[1], 1)
                          + chunks(v_sb[1], v_d[1], 1)
                          + chunks(kt_sb[2], kt_d[2], 1)
                          + chunks(qt_sb[2], qt_d[2], 1)
                          + chunks(v_sb[2], v_d[2], 1))
            for sb_t, dr in sync_loads:
                nc.sync.dma_start(out=sb_t, in_=dr)

            # ---------------- views
            qt_r, kt_r, v_r = [], [], []
            for g in range(3):
                nu = GNU[g]
                if g < 2:
                    qt_r.append(qt_sb[g].rearrange("p (i l) -> p i l", i=4))
                    kt_r.append(kt_sb[g].rearrange("p (c l) -> p c l", c=CB))
                else:
                    qt_r.append(qt_sb[g].rearrange("p (h i l) -> p h i l",
                                                   h=2, i=4))
                    kt_r.append(kt_sb[g].rearrange("p (h c l) -> p h c l",
                                                   h=2, c=CB))
                v_r.append(v_sb[g].rearrange("p (h c u x) -> p h c u x",
                                             h=2, c=CB, u=nu))

            # ---------------- staging for outputs
            stage = [inp.tile([128, 4, 2, GNU[g], 64], f32, tag=f"stage{g}",
                              name=f"stage{g}") for g in range(3)]

            # ---------------- sweeps: baseline S/exp structure (per-cb
            # 2-bank PSUM tiles, bank-aligned, h-pair concurrent-safe)
            # with the v2 normalize (recip + mul + add-tree) and staged
            # output DMA.
            at_tiles = {}
            sweeps = []
            for g in range(3):
                nu = GNU[g]
                sweeps += [(g, up) for up in
                           ([(0, 1), (2, 3)] if nu == 4 else
                            ([(0, 1)] if nu == 2 else [(0,)]))]
            done_units = set()

            def row_block(g, up, i):
                nu = GNU[g]
                nw = len(up)
                for h in range(2):
                    po = ps_o.tile([128, 2, 512], f32, tag="po", name="po")
                    for d in range(5):
                        ccb = i + d
                        ci0, _ = _rows_for_cb(ccb)
                        for us, u in enumerate(up):
                            a_t, aoff = at_tiles[(g, ccb, u)]
                            o0 = aoff + (i - ci0) * 128
                            lhsT = a_t[:, h, o0:o0 + 128]
                            rhs = v_r[g][:, h, ccb, u, 0:65]
                            nc.tensor.matmul(po[:, us, d * 66:d * 66 + 65],
                                             lhsT, rhs, start=True, stop=True)
                    pv = po[:, :, 0:330].rearrange("p w (c x) -> p w c x", x=66)
                    rc = rcp.tile([128, 2, 8], f32, tag="rc", name="rc")
                    nc.vector.reciprocal_approx_fast(
                        out=rc[:, 0:nw, 0:5], in_=pv[:, 0:nw, 0:5, 64])
                    on_gps = g == 0 or (g == 1 and h == 0)
                    tdt = f32 if on_gps else avdt
                    gi = 0 if on_gps else 1
                    tmp = tmpp.tile([128, 2, 5, 64], tdt, tag=f"tmp{gi}",
                                    name="tmp")
                    nc.vector.tensor_mul(
                        tmp[:, 0:nw], pv[:, 0:nw, 0:5, 0:64],
                        rc[:, 0:nw, 0:5].broadcast_to([128, nw, 5, 64]))
                    pair = trp.tile([128, 2, 2, 64], tdt, tag=f"pair{gi}",
                                    name="pair")
                    s2 = trp.tile([128, 2, 64], tdt, tag=f"s2{gi}", name="s2")
                    eng = nc.gpsimd if on_gps else nc.vector
                    t4 = tmp[:, 0:nw, 0:4, :].rearrange(
                        "p w (a b) x -> p w a b x", a=2)
                    eng.tensor_tensor(pair[:, 0:nw], t4[:, :, :, 0, :],
                                      t4[:, :, :, 1, :], add_op)
                    eng.tensor_tensor(s2[:, 0:nw], pair[:, 0:nw, 0, :],
                                      pair[:, 0:nw, 1, :], add_op)
                    eng.tensor_tensor(stage[g][:, i, h, up[0]:up[0] + nw, :],
                                      s2[:, 0:nw], tmp[:, 0:nw, 4, :], add_op)
                for u in up:
                    done_units.add((g, u, i))
                half = i // 2
                rows = (half * 2, half * 2 + 1)
                if all((g, uu, ii) in done_units
                       for uu in range(nu) for ii in rows):
                    w = 2 * 2 * nu * 64
                    nc.sync.dma_start(
                        out=out_d[g][:, half * w:(half + 1) * w],
                        in_=stage[g][:, half * 2:half * 2 + 2, :, :, :])

            # cb pairs (0,1) and (6,7) fit one 2-bank tile per head
            # (nr sums to 3 <= 4) and share a single ACTIVATE; adjacent
            # matmuls still alternate banks (h0 bank0 / h1 bank1), so the
            # concurrent-pair bank rule holds.
            CBGROUPS = [(0, 1), (2,), (3,), (4,), (5,), (6, 7)]
            for g, up in sweeps:
                C = GC[g]
                for grp in CBGROUPS:
                    for u in up:
                        sp = ps_s.tile([128, 2, 512], f32, tag="s", name="sp")
                        a_t = atp.tile([128, 2, 512], avdt, tag="at", name="at")
                        off = 0
                        for cb in grp:
                            i0, i1 = _rows_for_cb(cb)
                            nr = i1 - i0 + 1
                            for h in range(2):
                                if g < 2:
                                    lhsT = kt_r[g][64 * h:64 * h + 64, cb,
                                                   u * 128:(u + 1) * 128]
                                    rhs = qt_r[g][64 * h:64 * h + 64,
                                                  i0:i1 + 1,
                                                  u * 128:(u + 1) * 128]
                                else:
                                    lhsT = kt_r[g][0:C, h, cb,
                                                   u * 128:(u + 1) * 128]
                                    rhs = qt_r[g][0:C, h, i0:i1 + 1,
                                                  u * 128:(u + 1) * 128]
                                nc.tensor.matmul(
                                    sp[:, h, off:off + nr * 128], lhsT, rhs,
                                    start=True, stop=True)
                            at_tiles[(g, cb, u)] = (a_t, off)
                            off += nr * 128
                        nc.scalar.activation(a_t[:, :, 0:off],
                                             sp[:, :, 0:off],
                                             exp_fn, scale=SCALE)
                    for cb in grp:
                        if cb >= 4:
                            row_block(g, up, cb - 4)

    nc.compile()
    _BASS_CACHE["nc"] = nc
    return nc


def _build_bass_cached():
    return _build_bass()


# ---------------------------------------------------------------- host pack
def _np_dtype(name):
    if name == "bf16":
        import ml_dtypes
        return ml_dtypes.bfloat16
    if name == "f16":
        return np.float16
    return np.float32


def _pack_inputs(q, k, v):
    """q,k,v: [1, 8192, 12, 64] fp32 -> list of 8 per-core input dicts."""
    q = np.asarray(q, dtype=np.float32)
    k = np.asarray(k, dtype=np.float32)
    v = np.asarray(v, dtype=np.float32)
    s_np = _np_dtype(S_DTYPE)
    av_np = _np_dtype(AV_DTYPE)
    qb = q.reshape(NB, BLOCK, H, D)
    kb = k.reshape(NB, BLOCK, H, D)
    vb = v.reshape(NB, BLOCK, H, D)

    sub = np.repeat(np.arange(4), 32)                      # [128]
    U = (sub[None, :] == np.arange(4)[:, None]).astype(np.float32)  # [4,128]

    in_maps = []
    for core in range(NCORES):
        hh, R = core // 4, core % 4
        m = {}
        for g in range(3):
            L, nu, C = GL[g], GNU[g], GC[g]
            pos = POS[g]
            if g < 2:
                qt = np.zeros((128, 4 * L), np.float32)
                kt = np.zeros((128, CB * L), np.float32)
            else:
                qt = np.zeros((C, 2 * 4 * L), np.float32)
                kt = np.zeros((C, 2 * CB * L), np.float32)
            va = np.zeros((128, 2 * CB * nu * 66), np.float32)
            for h in range(2):
                head = 4 * g + 2 * hh + h
                for i in range(ROWS_PER_CORE):
                    r = 4 * R + i
                    tok = qb[r, pos, head, :]              # [L, 64]
                    if g < 2:
                        qt[64 * h:64 * h + 64, i * L:(i + 1) * L] = tok.T
                    else:
                        o = (h * 4 + i) * L
                        qt[0:64, o:o + L] = tok.T
                        qt[64, o:o + L] = -MASK_M
                        qt[65:69, o:o + L] = MASK_M * U
                for cb in range(CB):
                    c = 4 * R - 2 + cb
                    if 0 <= c < NB:
                        tok = kb[c, pos, head, :]          # [L, 64]
                        vt = vb[c, pos, head, :]           # [L, 64]
                    else:
                        tok = np.zeros((L, D), np.float32)
                        vt = np.zeros((L, D), np.float32)
                    if g < 2:
                        kt[64 * h:64 * h + 64, cb * L:(cb + 1) * L] = tok.T
                    else:
                        o = (h * CB + cb) * L
                        kt[0:64, o:o + L] = tok.T
                        kt[64, o:o + L] = 1.0
                        kt[65:69, o:o + L] = U
                    for u in range(nu):
                        o = ((h * CB + cb) * nu + u) * 66
                        va[:, o:o + 64] = vt[u * 128:(u + 1) * 128, :]
                        va[:, o + 64] = 1.0
            m[f"qt{g}"] = qt.astype(s_np)
            m[f"kt{g}"] = kt.astype(s_np)
            m[f"v{g}"] = va.astype(av_np)
        in_maps.append(m)
    return in_maps


def _unpack(results):
    out = np.zeros((B, S, H, D), np.float32)
    for core in range(NCORES):
        hh, R = core // 4, core % 4
        res = results[core]
        for g in range(3):
            L, nu = GL[g], GNU[g]
            pos = POS[g]
            og = np.asarray(res[f"out{g}"], dtype=np.float32)
            for h in range(2):
                head = 4 * g + 2 * hh + h
                for i in range(ROWS_PER_CORE):
                    r = 4 * R + i
                    for u in range(nu):
                        off = ((i * 2 + h) * nu + u) * 64
                        out[0, r * 512 + pos[u * 128:(u + 1) * 128], head, :] = \
                            og[:, off:off + 64]
    return out


# ---------------------------------------------------------------- entry
def _run(q, k, v, trace=False):
    from concourse.bass_utils import run_bass_kernel_spmd
    nc = _build_bass_cached()
    in_maps = _pack_inputs(q, k, v)
    res = run_bass_kernel_spmd(nc, in_maps, core_ids=list(range(NCORES)),
                               trace=trace)
    return _unpack(res.results), res


def kernel(q, k, v):
    out, _ = _run(q, k, v, trace=False)
    return out
